# revision 22
# baseline (speedup 1.0000x reference)
"""GCN encoder (3-layer) as a Bass/Tile kernel on 8 trn2 cores.

Math: PyG GCNConv on a batch of B=4 graphs sharing one edge set.
    deg/norm depend only on edge_index, so the message passing
        agg = segment_sum(norm * (h @ W)[src] -> dst)
    is exactly  A @ (h @ W)  with the dense normalized adjacency
        A[i, j] = sum_{e: dst=i, src=j} norm[e].

Fast path (the actual graph): edge_index is all-pairs + one extra self
loop per node, so deg == N+1 everywhere and A == c * (J + I) with
c = 1/(N+1).  Each layer is then
        z = c*(h W + 1 S) + b,   S = colsum(h W) = t W,  t = colsum(h)
          = (h + 1 t^T) (cW) + b
so the global-sum term never needs the dense adjacency: layer 2 takes
it as a per-partition relu bias from a tiny S-matmul (t1 @ cW2) that
hides under the big matmul, and layer 3 folds it as a per-partition
scalar add on h2 (only the core's own node half).  Layer 1 folds through
the rank-3 input:  h0 = x W0 + 1 b0  (x is [N,2]) gives
        z1 = (x + 1 xsum^T) G + 1 row1,  G = c W0 Ws0,
        row1 = Ws0^T b0 + bs0,           xsum = colsum(x)
with G/row1 precomputed on the host (weights-only folding; xsum is a
[2]-vector of input column sums).  Biases ride into the matmuls as an
extra all-ones contraction row, so layer biases that vary along the
free dim never need broadcasting.

Precision split (rel-err gate 2e-2): feat = h0 has near-zero entries,
so the h0 path stays exact fp32 (PE LOW_HIGH).  upd has |.| >= 2.9 and
tolerates bf16 chain matmuls (simulated max_rel ~5e-4), which run
single-pass on the PE at 4x fp32 throughput.

Last layer is emitted node-major directly (lhsT = h2 column blocks), so
log_softmax reduces along the free axis with no PE transposes.

Sharding: 2 cores per graph.  Both cores of a pair run the (cheap)
chain redundantly; the host packs each core's node half FIRST (the
chain is permutation-equivariant), so one SPMD program lets core
half h emit output rows [256h, 256h+256) from columns [0, 256).

General fallback (any other edge_index): build A on the host, run the
dense-matmul formulation (A.T chunks as matmul rhs/lhsT).
"""

import numpy as np

N = 512
B = 4
D = 2  # raw coord dim
H = 128  # embedding dim
L = 3
P = 128
NB = N // P  # node blocks (general path)
NH = 2  # node blocks per core half (structured path)
NUM_CORES = 8

_PROGRAM_CACHE = {}

# bfin pack: [ G+row1 | xs2T+ones ]  (3 real rows)
_BF_G = 0
_BF_X = H
_BF_COLS = H + N
# xpk pack: [ xT+ones | W0+b0 ]  (3 real rows)
_XP_XT = 0
_XP_W0 = N
_XP_COLS = N + H


def _patch_act_tables():
    """Point the compiler at an act-table root where the only set holding
    exp/ln is natural_log_exp_and_others. The stock lookup first-matches
    exp -> exp_and_others and ln -> natural_log, so an exp...ln kernel pays
    a ~1.3us mid-kernel ACT_TABLE_LOAD to switch sets; with the combined
    set loaded once at startup there are zero mid-kernel switches."""
    if _PROGRAM_CACHE.get("act_patched"):
        return
    try:
        import glob
        import json
        import os
        import tempfile

        import neuronxcc
        from neuronxcc.driver.jobs.support import FindActInfo

        pkg = os.path.dirname(neuronxcc.__file__)
        src_dir = os.path.join(pkg, "pwp", "pwp_bin_trainium")
        src_json = os.path.join(src_dir, "act_info.json")
        if not os.path.exists(src_json):
            return
        info = json.load(open(src_json))
        names = {s["name"] for s in info.get("act_func_sets", [])}
        if "natural_log_exp_and_others" not in names:
            return
        keep = [s for s in info["act_func_sets"]
                if s["name"] not in ("exp_and_others", "natural_log",
                                     "exp_and_friends")]
        keep.sort(key=lambda s: s["name"] != "natural_log_exp_and_others")
        info["act_func_sets"] = keep
        dst = tempfile.mkdtemp(prefix="act_root_")
        for f in glob.glob(os.path.join(src_dir, "*")):
            base = os.path.basename(f)
            if base != "act_info.json":
                os.symlink(f, os.path.join(dst, base))
        dst_json = os.path.join(dst, "act_info.json")
        json.dump(info, open(dst_json, "w"))

        orig = FindActInfo.findActInfoFile

        def patched(package_dir, arch):
            path = orig(package_dir, arch)
            if os.path.basename(os.path.dirname(path)) == "pwp_bin_trainium":
                return dst_json
            return path

        FindActInfo.findActInfoFile = patched
        from neuronxcc.driver.jobs import WalrusDriver
        if getattr(WalrusDriver, "findActInfoFile", None) is not None:
            WalrusDriver.findActInfoFile = patched
        _PROGRAM_CACHE["act_patched"] = True
    except Exception:
        pass  # fall back to the stock tables (one extra table load)


def _build_structured_program(bias_zero):
    import concourse.mybir as mybir
    import concourse.tile as tile
    from concourse import bacc
    from concourse import bass as bass_mod
    from contextlib import ExitStack

    f32 = mybir.dt.float32
    bf16 = mybir.dt.bfloat16
    AF = mybir.ActivationFunctionType
    OP = mybir.AluOpType
    AX = mybir.AxisListType

    # Skip the ~1us init all-engine barrier Bacc emits after its const
    # memsets: the only instructions here that read the const tiles before
    # the first real sync point are the warm-exp and the keep-alive
    # matmuls, whose outputs are never consumed, and every real const read
    # (activation bias pointers) happens microseconds after the gpsimd
    # memsets retire.
    orig_barrier = bass_mod.Bass.all_engine_barrier
    bass_mod.Bass.all_engine_barrier = (
        lambda self, *, sem_only=False: None)
    try:
        nc = bacc.Bacc("TRN2", target_bir_lowering=False, debug=False,
                       num_devices=NUM_CORES)
    finally:
        bass_mod.Bass.all_engine_barrier = orig_barrier

    bfin = nc.dram_tensor("bfin", [32, _BF_COLS], bf16,
                          kind="ExternalInput").ap()
    bpk = None
    if not bias_zero:
        bpk = nc.dram_tensor("bpk", [P, 1 + H], bf16,
                             kind="ExternalInput").ap()
    wpk = nc.dram_tensor("wpk", [P, 2 * H], bf16, kind="ExternalInput").ap()
    xpk = nc.dram_tensor("xpk", [32, _XP_COLS], f32,
                         kind="ExternalInput").ap()
    w0pk = nc.dram_tensor("w0pk", [32, H], f32, kind="ExternalInput").ap()

    updh = nc.dram_tensor("updh", [NH, P, H], f32, kind="ExternalOutput").ap()
    feath = nc.dram_tensor("feath", [NH, P, H], f32,
                           kind="ExternalOutput").ap()

    with tile.TileContext(nc) as tc, ExitStack() as ctx:
        const = ctx.enter_context(tc.tile_pool(name="const", bufs=1))
        hpool = ctx.enter_context(tc.tile_pool(name="hpool", bufs=4))
        work = ctx.enter_context(tc.tile_pool(name="work", bufs=4))
        stat = ctx.enter_context(tc.tile_pool(name="stat", bufs=8))
        psumB = ctx.enter_context(tc.tile_pool(name="psumB", bufs=2,
                                               space="PSUM"))
        psumZ = ctx.enter_context(tc.tile_pool(name="psumZ", bufs=1,
                                               space="PSUM"))
        psumH = ctx.enter_context(tc.tile_pool(name="psumH", bufs=1,
                                               space="PSUM"))
        psumS = ctx.enter_context(tc.tile_pool(name="psumS", bufs=1,
                                               space="PSUM"))
        psumK = ctx.enter_context(tc.tile_pool(name="psumK", bufs=1,
                                               space="PSUM"))

        bf_s = const.tile([P, _BF_COLS], bf16)
        xp_s = const.tile([P, _XP_COLS], f32)
        bp_s = const.tile([P, 1 + H], bf16)
        wp_s = const.tile([P, 2 * H], bf16)

        # Input DMAs issue first; packs carry 32 host-zeroed rows and every
        # input matmul contracts over K=32 only, so rows 32-127 are never
        # touched and no pad memsets exist at all.
        nc.sync.dma_start(out=bf_s[:32, :], in_=bfin[:])
        nc.sync.dma_start(out=wp_s[:], in_=wpk[:])
        if not bias_zero:
            nc.sync.dma_start(out=bp_s[:], in_=bpk[:])
        nc.gpsimd.dma_start(out=xp_s[:32, :], in_=xpk[:])
        # W0/b0 ship separately and deliberately late (2nd SWDGE issue):
        # the static PE schedule greedily hoists the fp32 h0 matmuls ahead
        # of z2 when their inputs are ready early, which on fast-DMA runs
        # stalls the chain by ~1us.  Landing W0 after z2 launches pins the
        # h0 blocks into the relu2 shadow where they belong.
        w0_t = const.tile([P, H], f32)
        nc.gpsimd.dma_start(out=w0_t[:32, :], in_=w0pk[:])

        # first ACT instruction: a throwaway Exp off the framework zero
        # const, so the (patched, combined exp+ln) table set loads during
        # the DMA window with no memset dependency.
        warm = stat.tile([P, 1], f32, tag="warm")
        zero_c = nc.const_aps.tensor(0.0, [P, 1], f32)
        nc.scalar.activation(warm[:], zero_c, AF.Exp)

        # HAM keep-alive: the PE clock reaches full speed only when an
        # activity window is busy enough.  Fill the ~3us DMA-wait gap
        # before z1 with 256-col filler matmuls (ending before the
        # earliest possible bfin arrival) so the chain runs up-clocked;
        # tiny const-fed matmuls cover the short mid-chain stalls.
        ones_c = nc.const_aps.tensor(1.0, [P, 1], bf16)
        ka_ps = psumK.tile([P, 2 * H], f32, tag="ka")
        kab = const.tile([P, 2 * H], bf16)
        nc.vector.memset(kab[:], 1.0)

        def keepalive(n):
            for _ in range(n):
                nc.tensor.matmul(ka_ps[0:1, 0:1], ones_c, ones_c,
                                 start=True, stop=True)

        def keepalive_big(n):
            for _ in range(n):
                nc.tensor.matmul(ka_ps[0:1, :], kab[:, 0:1], kab[:],
                                 start=True, stop=True)

        G_s = bf_s[:32, _BF_G:_BF_G + H]
        xs2_s = bf_s[:32, _BF_X:_BF_X + N]
        b1T_s = bp_s[:, 0:1] if not bias_zero else 0.0
        b2bc_s = bp_s[:, 1:1 + H] if not bias_zero else None
        w2_s = wp_s[:, 0:H]
        w3_s = wp_s[:, H:2 * H]
        xt_s = xp_s[:32, _XP_XT:_XP_XT + N]
        w0_s = w0_t[:32, :]

        # ---- layer 1 (rank-3 folded): z1 = xs2 @ G + 1 row1 ----
        keepalive_big(5)
        z1_ps = psumB.tile([P, N], f32, tag="big")
        nc.tensor.matmul(z1_ps[:], G_s, xs2_s, start=True, stop=True)
        keepalive(2)
        h1 = hpool.tile([P, N], bf16, tag="h")
        t1 = stat.tile([P, 1], f32, tag="t")
        nc.scalar.activation(h1[:], z1_ps[:], AF.Relu, accum_out=t1[:])

        # ---- layer 2: z2 = h1 @ (c W2) + 1 (t1 (c W2)) + 1 b1 ----
        # The global-sum term rides as a per-partition bias (dim-major), so
        # the big matmul starts the moment relu1 retires, with no h+t pass
        # in between; the tiny S-matmul and bias copy run in its shadow.
        z2_ps = psumB.tile([P, N], f32, tag="big")
        h0_ps = psumH.tile([P, NH, H], f32, tag="h0")
        bias2 = stat.tile([P, 1], f32, tag="b2")
        if bias_zero:
            t1b = stat.tile([P, 1], bf16, tag="tb")
            nc.vector.tensor_copy(out=t1b[:], in_=t1[:])
            nc.tensor.matmul(z2_ps[:], w2_s, h1[:], start=True, stop=True)
            s2_ps = psumS.tile([P, 1], f32, tag="s")
            nc.tensor.matmul(s2_ps[:], w2_s, t1b[:], start=True, stop=True)
            nc.vector.tensor_copy(out=bias2[:], in_=s2_ps[:])
        else:
            h1t = hpool.tile([P, N], bf16, tag="h")
            nc.vector.tensor_scalar_add(out=h1t[:], in0=h1[:],
                                        scalar1=t1[:, 0:1])
            nc.tensor.matmul(z2_ps[:], w2_s, h1t[:], start=True, stop=True)
        # h0 block 0 (exact fp32; bias b0 rides in as the ones row) fills
        # the PE gap under relu2
        nc.tensor.matmul(h0_ps[:, 0, :], xt_s[:, 0:P], w0_s,
                         start=True, stop=True)
        h2 = hpool.tile([P, N], bf16, tag="h")
        t2 = stat.tile([P, 1], f32, tag="t")
        if bias_zero:
            nc.scalar.activation(h2[:], z2_ps[:], AF.Relu,
                                 bias=bias2[:, 0:1], accum_out=t2[:])
        else:
            nc.scalar.activation(h2[:], z2_ps[:], AF.Relu, bias=b1T_s,
                                 accum_out=t2[:])

        # ---- layer 3, node-major: z3[j] = (h2 + 1 t2^T)[:,j].T @ (c W3);
        # only this core's node half feeds it, the rest of h2 is consumed
        # through t2 alone
        h2t = hpool.tile([P, NH * H], bf16, tag="h2t")
        nc.vector.tensor_scalar_add(out=h2t[:, 0:H], in0=h2[:, 0:H],
                                    scalar1=t2[:, 0:1])
        nc.vector.tensor_scalar_add(out=h2t[:, H:2 * H], in0=h2[:, H:2 * H],
                                    scalar1=t2[:, 0:1])
        z3_ps = psumZ.tile([P, NH, H], f32, tag="z3")
        nc.tensor.matmul(z3_ps[:, 0, :], h2t[:, 0:P], w3_s,
                         start=True, stop=True)
        nc.tensor.matmul(z3_ps[:, 1, :], h2t[:, P:2 * P], w3_s,
                         start=True, stop=True)
        # h0 block 1 on the now-idle PE; its consumers (p1, feat copy) are
        # late in the tail
        nc.tensor.matmul(h0_ps[:, 1, :], xt_s[:, P:2 * P], w0_s,
                         start=True, stop=True)

        if bias_zero:
            ze = z3_ps  # exp straight off PSUM
        else:
            ze = work.tile([P, NH, H], f32, tag="z3b")
            nc.vector.tensor_add(out=ze[:, 0, :], in0=z3_ps[:, 0, :],
                                 in1=b2bc_s)
            nc.vector.tensor_add(out=ze[:, 1, :], in0=z3_ps[:, 1, :],
                                 in1=b2bc_s)

        # h0 -> SBUF copies (a tensor_tensor may read only ONE PSUM
        # operand, so the residual add needs h0 in SBUF): block 0 on the
        # scalar engine under the z3 matmuls, block 1 on the DVE right
        # before its use.
        h0b = work.tile([P, NH, H], f32, tag="h0b")
        nc.scalar.copy(out=h0b[:, 0, :], in_=h0_ps[:, 0, :])

        # log_softmax along the free axis + residual; values are O(+-10),
        # so exp() without max-subtraction is safe.  p = z3 + h0 overlaps
        # the exp; -lse = Ln(1/sum) so block 0 adds it as an activation
        # bias on scalar while block 1 adds it on DVE.
        e = work.tile([P, NH, H], bf16, tag="e")
        nc.scalar.activation(e[:], ze[:], AF.Exp)
        p = work.tile([P, NH, H], f32, tag="p")
        nc.vector.tensor_add(out=p[:, 0, :], in0=ze[:, 0, :],
                             in1=h0b[:, 0, :])
        ssum = stat.tile([P, NH], f32, tag="ssum")
        nc.vector.reduce_sum(ssum[:], e[:], axis=AX.X)
        rs = stat.tile([P, NH], f32, tag="rs")
        nc.vector.reciprocal(out=rs[:], in_=ssum[:])
        nc.vector.tensor_copy(out=h0b[:, 1, :], in_=h0_ps[:, 1, :])
        nc.vector.tensor_add(out=p[:, 1, :], in0=ze[:, 1, :],
                             in1=h0b[:, 1, :])
        nlse = stat.tile([P, NH], f32, tag="nlse")
        nc.scalar.activation(nlse[:], rs[:], AF.Ln)

        o = work.tile([P, NH, H], f32, tag="o")
        nc.scalar.activation(o[:, 0, :], p[:, 0, :], AF.Identity,
                             bias=nlse[:, 0:1])
        nc.scalar.dma_start(out=updh[0], in_=o[:, 0, :])
        nc.vector.tensor_scalar_add(out=o[:, 1, :], in0=p[:, 1, :],
                                    scalar1=nlse[:, 1:2])
        nc.sync.dma_start(out=updh[1], in_=o[:, 1, :])

        nc.gpsimd.dma_start(out=feath[:].rearrange("b p f -> p b f"),
                            in_=h0b[:])

        # consume the keep-alive PSUM so the filler matmuls survive DCE
        nc.vector.tensor_copy(out=warm[0:1, :], in_=ka_ps[0:1, 0:1])

    nc.compile()
    return nc


def _build_general_program():
    """Arbitrary edge_index: dense normalized adjacency as matmuls."""
    import concourse.mybir as mybir
    import concourse.tile as tile
    from concourse import bacc
    from contextlib import ExitStack

    f32 = mybir.dt.float32
    AF = mybir.ActivationFunctionType
    AX = mybir.AxisListType

    nc = bacc.Bacc("TRN2", target_bir_lowering=False, debug=False,
                   num_devices=NUM_CORES)

    xTp = nc.dram_tensor("xTp", [P, N], f32, kind="ExternalInput").ap()
    w0p = nc.dram_tensor("w0p", [P, H], f32, kind="ExternalInput").ap()
    b0T = nc.dram_tensor("b0T", [P, 1], f32, kind="ExternalInput").ap()
    b0bc = nc.dram_tensor("b0bc", [P, H], f32, kind="ExternalInput").ap()
    wsT = nc.dram_tensor("wsT", [P, L, H], f32, kind="ExternalInput").ap()
    bsT = nc.dram_tensor("bsT", [P, L], f32, kind="ExternalInput").ap()
    bs2bc = nc.dram_tensor("bs2bc", [P, H], f32, kind="ExternalInput").ap()
    at = nc.dram_tensor("at", [P, NB, N], f32, kind="ExternalInput").ap()

    upd = nc.dram_tensor("upd", [N, H], f32, kind="ExternalOutput").ap()
    feat = nc.dram_tensor("feat", [N, H], f32, kind="ExternalOutput").ap()

    with tile.TileContext(nc) as tc, ExitStack() as ctx:
        const = ctx.enter_context(tc.tile_pool(name="const", bufs=1))
        hpool = ctx.enter_context(tc.tile_pool(name="hpool", bufs=2))
        work = ctx.enter_context(tc.tile_pool(name="work", bufs=2))
        zpool = ctx.enter_context(tc.tile_pool(name="zpool", bufs=4))
        stat = ctx.enter_context(tc.tile_pool(name="stat", bufs=8))
        psum = ctx.enter_context(tc.tile_pool(name="psum", bufs=3, space="PSUM"))
        psumB = ctx.enter_context(tc.tile_pool(name="psumB", bufs=2, space="PSUM"))

        warm = stat.tile([P, 1], f32, tag="warm")
        nc.vector.memset(warm[:], 1.0)
        nc.scalar.activation(warm[:], warm[:], AF.Ln)

        xT_s = const.tile([P, N], f32)
        nc.sync.dma_start(out=xT_s[:], in_=xTp[:])
        w0_s = const.tile([P, H], f32)
        nc.sync.dma_start(out=w0_s[:], in_=w0p[:])
        ws_s = const.tile([P, L, H], f32)
        nc.sync.dma_start(out=ws_s[:], in_=wsT[:])
        b0T_s = const.tile([P, 1], f32)
        nc.sync.dma_start(out=b0T_s[:], in_=b0T[:])
        bsT_s = const.tile([P, L], f32)
        nc.sync.dma_start(out=bsT_s[:], in_=bsT[:])
        b0bc_s = const.tile([P, H], f32)
        nc.sync.dma_start(out=b0bc_s[:], in_=b0bc[:])
        bs2bc_s = const.tile([P, H], f32)
        nc.sync.dma_start(out=bs2bc_s[:], in_=bs2bc[:])
        at_s = const.tile([P, NB, N], f32)
        nc.sync.dma_start(out=at_s[:], in_=at[:])

        h0T_ps = psumB.tile([P, N], f32, tag="big")
        nc.tensor.matmul(h0T_ps[:], w0_s[:], xT_s[:], start=True, stop=True)
        hT = hpool.tile([P, N], f32, tag="hT")
        nc.vector.tensor_scalar_add(out=hT[:], in0=h0T_ps[:],
                                    scalar1=b0T_s[:, 0:1])

        h0_s = const.tile([P, NB, H], f32)
        for b in range(NB):
            ps = psum.tile([P, H], f32, tag="mm")
            nc.tensor.matmul(ps[:], xT_s[:, b * P:(b + 1) * P], w0_s[:],
                             start=True, stop=True)
            nc.vector.tensor_add(out=h0_s[:, b, :], in0=ps[:], in1=b0bc_s[:])
            nc.sync.dma_start(out=feat[b * P:(b + 1) * P, :], in_=h0_s[:, b, :])

        for l in range(L):
            hw_s = work.tile([P, NB, H], f32, tag="hw")
            for b in range(NB):
                ps = psum.tile([P, H], f32, tag="mm")
                nc.tensor.matmul(ps[:], hT[:, b * P:(b + 1) * P],
                                 ws_s[:, l, :], start=True, stop=True)
                nc.vector.tensor_copy(out=hw_s[:, b, :], in_=ps[:])

            if l < L - 1:
                aggT_ps = psumB.tile([P, N], f32, tag="big")
                for cc in range(NB):
                    nc.tensor.matmul(aggT_ps[:], hw_s[:, cc, :], at_s[:, cc, :],
                                     start=(cc == 0), stop=(cc == NB - 1))
                hT_new = hpool.tile([P, N], f32, tag="hT")
                nc.scalar.activation(hT_new[:], aggT_ps[:], AF.Relu,
                                     bias=bsT_s[:, l:l + 1])
                hT = hT_new
            else:
                z_s = []
                negm_s = []
                s_sum = stat.tile([P, NB], f32, tag="ssum")
                for b in range(NB):
                    agg_ps = psum.tile([P, H], f32, tag="mm")
                    for cc in range(NB):
                        nc.tensor.matmul(agg_ps[:],
                                         at_s[:, cc, b * P:(b + 1) * P],
                                         hw_s[:, cc, :],
                                         start=(cc == 0), stop=(cc == NB - 1))
                    z = zpool.tile([P, H], f32, tag="z")
                    nc.vector.tensor_add(out=z[:], in0=agg_ps[:], in1=bs2bc_s[:])
                    negm = stat.tile([P, 1], f32, tag="negm")
                    nc.vector.reduce_max(negm[:], z[:], axis=AX.X, negate=True)
                    z_s.append(z)
                    negm_s.append(negm)
                for b in range(NB):
                    e = zpool.tile([P, H], f32, tag="e")
                    nc.scalar.activation(e[:], z_s[b][:],
                                         mybir.ActivationFunctionType.Exp,
                                         bias=negm_s[b][:, 0:1],
                                         accum_out=s_sum[:, b:b + 1])
                lse = stat.tile([P, NB], f32, tag="lse")
                nc.scalar.activation(lse[:], s_sum[:],
                                     mybir.ActivationFunctionType.Ln)
                for b in range(NB):
                    tot = stat.tile([P, 1], f32, tag="tot")
                    nc.vector.tensor_sub(out=tot[:], in0=lse[:, b:b + 1],
                                         in1=negm_s[b][:])
                    o = zpool.tile([P, H], f32, tag="o")
                    nc.vector.scalar_tensor_tensor(
                        out=o[:], in0=z_s[b][:], scalar=tot[:, 0:1],
                        in1=h0_s[:, b, :],
                        op0=mybir.AluOpType.subtract, op1=mybir.AluOpType.add)
                    nc.sync.dma_start(out=upd[b * P:(b + 1) * P, :], in_=o[:])

    nc.compile()
    return nc


def _edge_structure(edge_index: np.ndarray):
    """Return True iff edge_index is exactly all-pairs + one self loop per
    node (uniform deg = N+1)."""
    src = edge_index[0].astype(np.int64)
    dst = edge_index[1].astype(np.int64)
    if src.shape[0] != N * N + N:
        return False
    if src.min() < 0 or src.max() >= N or dst.min() < 0 or dst.max() >= N:
        return False
    counts = np.bincount(src * N + dst, minlength=N * N).reshape(N, N)
    expect = np.ones((N, N), dtype=counts.dtype)
    np.fill_diagonal(expect, 2)
    return np.array_equal(counts, expect)


def _build_adjacency(edge_index: np.ndarray) -> np.ndarray:
    """Dense normalized adjacency, transposed: AT[src, dst] (= A.T)."""
    src = edge_index[0].astype(np.int64)
    dst = edge_index[1].astype(np.int64)
    deg = np.bincount(dst, minlength=N).astype(np.float32)
    dinv = np.where(deg > 0, 1.0 / np.sqrt(deg), 0.0).astype(np.float32)
    norm = (dinv[src] * dinv[dst]).astype(np.float32)
    at = np.bincount(src * N + dst, weights=norm.astype(np.float64),
                     minlength=N * N).reshape(N, N)
    return at.astype(np.float32)


def _pad_rows(a: np.ndarray, rows: int) -> np.ndarray:
    out = np.zeros((rows,) + a.shape[1:], dtype=a.dtype)
    out[:a.shape[0]] = a
    return out


def _structured_packs(x, W0, b0, Ws, bs, bias_zero=False):
    """Host-side constant folding (weights in fp64, rounded once)."""
    import ml_dtypes
    bf = ml_dtypes.bfloat16
    c = 1.0 / (N + 1)
    W0_64 = W0.astype(np.float64)
    Ws_64 = Ws.astype(np.float64)
    G = c * (W0_64 @ Ws_64[0])                       # [2, H]
    row1 = Ws_64[0].T @ b0.astype(np.float64) + bs[0]  # [H]

    bpk = np.zeros((P, 1 + H), dtype=bf)
    bpk[:, 0] = bs[1].astype(bf)
    bpk[:, 1:] = np.broadcast_to(bs[2], (P, H)).astype(bf)

    wpk = np.zeros((P, 2 * H), dtype=bf)
    wpk[:, :H] = (c * Ws_64[1]).astype(bf)
    wpk[:, H:] = (c * Ws_64[2]).astype(bf)

    w0p = np.zeros((32, H), dtype=np.float32)
    w0p[:D] = W0
    w0p[D] = b0
    shared = {"wpk": wpk, "w0pk": w0p} if bias_zero \
        else {"bpk": bpk, "wpk": wpk, "w0pk": w0p}

    per_core = []
    for core in range(NUM_CORES):
        g = core % B
        half = core // B
        hn = N // 2
        mine = np.arange(half * hn, (half + 1) * hn)
        other = np.arange(0, half * hn)
        rest = np.arange((half + 1) * hn, N)
        perm = np.concatenate([mine, other, rest])  # this core's half first
        x64 = x[g].astype(np.float64)
        xs2 = (x64 + x64.sum(0))[perm]               # [N, 2]

        bfin = np.zeros((32, _BF_COLS), dtype=bf)
        bfin[:D, _BF_G:_BF_G + H] = G.astype(bf)
        bfin[D, _BF_G:_BF_G + H] = row1.astype(bf)
        bfin[:D, _BF_X:_BF_X + N] = xs2.T.astype(bf)
        bfin[D, _BF_X:_BF_X + N] = 1.0

        xpk = np.zeros((32, _XP_COLS), dtype=np.float32)
        xpk[:D, _XP_XT:_XP_XT + N] = x[g][perm].T
        xpk[D, _XP_XT:_XP_XT + N] = 1.0
        xpk[:D, _XP_W0:_XP_W0 + H] = W0
        xpk[D, _XP_W0:_XP_W0 + H] = b0
        m = dict(shared)
        m["bfin"] = bfin
        m["xpk"] = xpk
        per_core.append(m)
    return per_core


def kernel(x, W0, b0, Ws, bs, edge_index):
    from concourse.bass_utils import run_bass_kernel_spmd

    _patch_act_tables()

    x = np.ascontiguousarray(np.asarray(x, dtype=np.float32))
    W0 = np.ascontiguousarray(np.asarray(W0, dtype=np.float32))
    b0 = np.ascontiguousarray(np.asarray(b0, dtype=np.float32))
    Ws = np.ascontiguousarray(np.asarray(Ws, dtype=np.float32))
    bs = np.ascontiguousarray(np.asarray(bs, dtype=np.float32))
    edge_index = np.asarray(edge_index, dtype=np.int32)

    structured = _edge_structure(edge_index)
    if structured:
        bias_zero = not (b0.any() or bs.any())
        in_maps = _structured_packs(x, W0, b0, Ws, bs, bias_zero)
        key = ("structured", bias_zero)
        if key not in _PROGRAM_CACHE:
            _PROGRAM_CACHE[key] = _build_structured_program(bias_zero)
        nc = _PROGRAM_CACHE[key]
    else:
        shared = {
            "w0p": _pad_rows(W0, P),
            "b0T": np.ascontiguousarray(b0.reshape(P, 1)),
            "b0bc": np.ascontiguousarray(np.broadcast_to(b0, (P, H))),
            "wsT": np.ascontiguousarray(Ws.transpose(1, 0, 2)),
            "bsT": np.ascontiguousarray(bs.T),
        }
        key = "general"
        if key not in _PROGRAM_CACHE:
            _PROGRAM_CACHE[key] = _build_general_program()
        nc = _PROGRAM_CACHE[key]
        at = _build_adjacency(edge_index)
        shared["at"] = np.ascontiguousarray(
            at.reshape(NB, P, N).transpose(1, 0, 2))
        shared["bs2bc"] = np.ascontiguousarray(
            np.broadcast_to(bs[L - 1], (P, H)))
        in_maps = []
        for core in range(NUM_CORES):
            g = core % B
            m = dict(shared)
            m["xTp"] = _pad_rows(np.ascontiguousarray(x[g].T), P)
            in_maps.append(m)

    res = run_bass_kernel_spmd(nc, in_maps, list(range(NUM_CORES)))
    _PROGRAM_CACHE["last_results"] = res

    if structured:
        upd = np.empty((B, N, H), dtype=np.float32)
        feat = np.empty((B, N, H), dtype=np.float32)
        for core in range(NUM_CORES):
            g = core % B
            half = core // B
            sl = slice(half * (N // 2), (half + 1) * (N // 2))
            upd[g, sl] = res.results[core]["updh"].reshape(N // 2, H)
            feat[g, sl] = res.results[core]["feath"].reshape(N // 2, H)
    else:
        upd = np.stack([res.results[g]["upd"] for g in range(B)])
        feat = np.stack([res.results[g]["feat"] for g in range(B)])
    return upd, feat


# revision 23
# speedup vs baseline: 1.0628x; 1.0628x over previous
"""GCN encoder (3-layer) as a Bass/Tile kernel on 8 trn2 cores.

Math: PyG GCNConv on a batch of B=4 graphs sharing one edge set.
    deg/norm depend only on edge_index, so the message passing
        agg = segment_sum(norm * (h @ W)[src] -> dst)
    is exactly  A @ (h @ W)  with the dense normalized adjacency
        A[i, j] = sum_{e: dst=i, src=j} norm[e].

Fast path (the actual graph): edge_index is all-pairs + one extra self
loop per node, so deg == N+1 everywhere and A == c * (J + I) with
c = 1/(N+1).  Each layer is then
        z = c*(h W + 1 S) + b,   S = colsum(h W) = t W,  t = colsum(h)
          = (h + 1 t^T) (cW) + b
so the global-sum term never needs the dense adjacency: layer 2 takes
it as a per-partition relu bias from a tiny S-matmul (t1 @ cW2) that
hides under the big matmul, and layer 3 folds it as a per-partition
scalar add on h2 (only the core's own node half).  Layer 1 folds through
the rank-3 input:  h0 = x W0 + 1 b0  (x is [N,2]) gives
        z1 = (x + 1 xsum^T) G + 1 row1,  G = c W0 Ws0,
        row1 = Ws0^T b0 + bs0,           xsum = colsum(x)
with G/row1 precomputed on the host (weights-only folding; xsum is a
[2]-vector of input column sums).  Biases ride into the matmuls as an
extra all-ones contraction row, so layer biases that vary along the
free dim never need broadcasting.

Precision split (rel-err gate 2e-2): feat = h0 has near-zero entries,
so the h0 path stays exact fp32 (PE LOW_HIGH).  upd has |.| >= 2.9 and
tolerates bf16 chain matmuls (simulated max_rel ~5e-4), which run
single-pass on the PE at 4x fp32 throughput.

Last layer is emitted node-major directly (lhsT = h2 column blocks), so
log_softmax reduces along the free axis with no PE transposes.

Sharding: 2 cores per graph.  Both cores of a pair run the (cheap)
chain redundantly; the host packs each core's node half FIRST (the
chain is permutation-equivariant), so one SPMD program lets core
half h emit output rows [256h, 256h+256) from columns [0, 256).

General fallback (any other edge_index): build A on the host, run the
dense-matmul formulation (A.T chunks as matmul rhs/lhsT).
"""

import numpy as np

N = 512
B = 4
D = 2  # raw coord dim
H = 128  # embedding dim
L = 3
P = 128
NB = N // P  # node blocks (general path)
NH = 2  # node blocks per core half (structured path)
NUM_CORES = 8

_PROGRAM_CACHE = {}

# bfin pack: [ G+row1 | xs2T+ones ]  (3 real rows)
_BF_G = 0
_BF_X = H
_BF_COLS = H + N
# xpk pack: [ xT+ones | W0+b0 ]  (3 real rows)
_XP_XT = 0
_XP_W0 = N
_XP_COLS = N + H


def _patch_act_tables():
    """Point the compiler at an act-table root where the only set holding
    exp/ln is natural_log_exp_and_others. The stock lookup first-matches
    exp -> exp_and_others and ln -> natural_log, so an exp...ln kernel pays
    a ~1.3us mid-kernel ACT_TABLE_LOAD to switch sets; with the combined
    set loaded once at startup there are zero mid-kernel switches."""
    if _PROGRAM_CACHE.get("act_patched"):
        return
    try:
        import glob
        import json
        import os
        import tempfile

        import neuronxcc
        from neuronxcc.driver.jobs.support import FindActInfo

        pkg = os.path.dirname(neuronxcc.__file__)
        src_dir = os.path.join(pkg, "pwp", "pwp_bin_trainium")
        src_json = os.path.join(src_dir, "act_info.json")
        if not os.path.exists(src_json):
            return
        info = json.load(open(src_json))
        names = {s["name"] for s in info.get("act_func_sets", [])}
        if "natural_log_exp_and_others" not in names:
            return
        keep = [s for s in info["act_func_sets"]
                if s["name"] not in ("exp_and_others", "natural_log",
                                     "exp_and_friends")]
        keep.sort(key=lambda s: s["name"] != "natural_log_exp_and_others")
        info["act_func_sets"] = keep
        dst = tempfile.mkdtemp(prefix="act_root_")
        for f in glob.glob(os.path.join(src_dir, "*")):
            base = os.path.basename(f)
            if base != "act_info.json":
                os.symlink(f, os.path.join(dst, base))
        dst_json = os.path.join(dst, "act_info.json")
        json.dump(info, open(dst_json, "w"))

        orig = FindActInfo.findActInfoFile

        def patched(package_dir, arch):
            path = orig(package_dir, arch)
            if os.path.basename(os.path.dirname(path)) == "pwp_bin_trainium":
                return dst_json
            return path

        FindActInfo.findActInfoFile = patched
        from neuronxcc.driver.jobs import WalrusDriver
        if getattr(WalrusDriver, "findActInfoFile", None) is not None:
            WalrusDriver.findActInfoFile = patched
        _PROGRAM_CACHE["act_patched"] = True
    except Exception:
        pass  # fall back to the stock tables (one extra table load)


def _build_structured_program(bias_zero):
    import concourse.mybir as mybir
    import concourse.tile as tile
    from concourse import bacc
    from concourse import bass as bass_mod
    from contextlib import ExitStack

    f32 = mybir.dt.float32
    bf16 = mybir.dt.bfloat16
    AF = mybir.ActivationFunctionType
    OP = mybir.AluOpType
    AX = mybir.AxisListType

    # Skip the ~1us init all-engine barrier Bacc emits after its const
    # memsets: the only instructions here that read the const tiles before
    # the first real sync point are the warm-exp and the keep-alive
    # matmuls, whose outputs are never consumed, and every real const read
    # (activation bias pointers) happens microseconds after the gpsimd
    # memsets retire.
    orig_barrier = bass_mod.Bass.all_engine_barrier
    bass_mod.Bass.all_engine_barrier = (
        lambda self, *, sem_only=False: None)
    try:
        nc = bacc.Bacc("TRN2", target_bir_lowering=False, debug=False,
                       num_devices=NUM_CORES)
    finally:
        bass_mod.Bass.all_engine_barrier = orig_barrier

    bfin = nc.dram_tensor("bfin", [32, _BF_COLS], bf16,
                          kind="ExternalInput").ap()
    bpk = None
    if not bias_zero:
        bpk = nc.dram_tensor("bpk", [P, 1 + H], bf16,
                             kind="ExternalInput").ap()
    wpk = nc.dram_tensor("wpk", [P, 2 * H], bf16, kind="ExternalInput").ap()
    xpk = nc.dram_tensor("xpk", [32, _XP_COLS], f32,
                         kind="ExternalInput").ap()

    updh = nc.dram_tensor("updh", [NH, P, H], f32, kind="ExternalOutput").ap()
    feath = nc.dram_tensor("feath", [NH, P, H], f32,
                           kind="ExternalOutput").ap()

    with tile.TileContext(nc) as tc, ExitStack() as ctx:
        const = ctx.enter_context(tc.tile_pool(name="const", bufs=1))
        hpool = ctx.enter_context(tc.tile_pool(name="hpool", bufs=4))
        work = ctx.enter_context(tc.tile_pool(name="work", bufs=4))
        stat = ctx.enter_context(tc.tile_pool(name="stat", bufs=8))
        psumB = ctx.enter_context(tc.tile_pool(name="psumB", bufs=2,
                                               space="PSUM"))
        psumZ = ctx.enter_context(tc.tile_pool(name="psumZ", bufs=1,
                                               space="PSUM"))
        psumH = ctx.enter_context(tc.tile_pool(name="psumH", bufs=1,
                                               space="PSUM"))
        psumS = ctx.enter_context(tc.tile_pool(name="psumS", bufs=1,
                                               space="PSUM"))
        psumK = ctx.enter_context(tc.tile_pool(name="psumK", bufs=1,
                                               space="PSUM"))

        bf_s = const.tile([P, _BF_COLS], bf16)
        xp_s = const.tile([P, _XP_COLS], f32)
        bp_s = const.tile([P, 1 + H], bf16)
        wp_s = const.tile([P, 2 * H], bf16)

        # Input DMAs issue first; packs carry 32 host-zeroed rows and every
        # input matmul contracts over K=32 only, so rows 32-127 are never
        # touched and no pad memsets exist at all.
        nc.sync.dma_start(out=bf_s[:32, :], in_=bfin[:])
        nc.sync.dma_start(out=wp_s[:], in_=wpk[:])
        if not bias_zero:
            nc.sync.dma_start(out=bp_s[:], in_=bpk[:])
        nc.gpsimd.dma_start(out=xp_s[:32, :], in_=xpk[:])

        # first ACT instruction: a throwaway Exp off the framework zero
        # const, so the (patched, combined exp+ln) table set loads during
        # the DMA window with no memset dependency.
        warm = stat.tile([P, 1], f32, tag="warm")
        zero_c = nc.const_aps.tensor(0.0, [P, 1], f32)
        nc.scalar.activation(warm[:], zero_c, AF.Exp)

        # HAM keep-alive: the PE clock reaches full speed only when an
        # activity window is busy enough.  Fill the ~3us DMA-wait gap
        # before z1 with 256-col filler matmuls (ending before the
        # earliest possible bfin arrival) so the chain runs up-clocked;
        # tiny const-fed matmuls cover the short mid-chain stalls.
        ones_c = nc.const_aps.tensor(1.0, [P, 1], bf16)
        ka_ps = psumK.tile([P, 2 * H], f32, tag="ka")
        kab = const.tile([P, 2 * H], bf16)
        nc.vector.memset(kab[:], 1.0)

        def keepalive(n):
            for _ in range(n):
                nc.tensor.matmul(ka_ps[0:1, 0:1], ones_c, ones_c,
                                 start=True, stop=True)

        def keepalive_big(n):
            for _ in range(n):
                nc.tensor.matmul(ka_ps[0:1, :], kab[:, 0:1], kab[:],
                                 start=True, stop=True)

        G_s = bf_s[:32, _BF_G:_BF_G + H]
        xs2_s = bf_s[:32, _BF_X:_BF_X + N]
        b1T_s = bp_s[:, 0:1] if not bias_zero else 0.0
        b2bc_s = bp_s[:, 1:1 + H] if not bias_zero else None
        w2_s = wp_s[:, 0:H]
        w3_s = wp_s[:, H:2 * H]
        xt_s = xp_s[:32, _XP_XT:_XP_XT + N]
        w0_s = xp_s[:32, _XP_W0:_XP_W0 + H]

        # ---- layer 1 (rank-3 folded): z1 = xs2 @ G + 1 row1 ----
        keepalive_big(5)
        z1_ps = psumB.tile([P, N], f32, tag="big")
        nc.tensor.matmul(z1_ps[:], G_s, xs2_s, start=True, stop=True)
        keepalive(2)
        h1 = hpool.tile([P, N], bf16, tag="h")
        t1 = stat.tile([P, 1], f32, tag="t")
        nc.scalar.activation(h1[:], z1_ps[:], AF.Relu, accum_out=t1[:])

        # ---- layer 2: z2 = h1 @ (c W2) + 1 (t1 (c W2)) + 1 b1 ----
        # The global-sum term rides as a per-partition bias (dim-major), so
        # the big matmul starts the moment relu1 retires, with no h+t pass
        # in between; the tiny S-matmul and bias copy run in its shadow.
        z2_ps = psumB.tile([P, N], f32, tag="big")
        h0_ps = psumH.tile([P, NH, H], f32, tag="h0")
        bias2 = stat.tile([P, 1], f32, tag="b2")
        if bias_zero:
            t1b = stat.tile([P, 1], bf16, tag="tb")
            nc.vector.tensor_copy(out=t1b[:], in_=t1[:])
            nc.tensor.matmul(z2_ps[:], w2_s, h1[:], start=True, stop=True)
            s2_ps = psumS.tile([P, 1], f32, tag="s")
            nc.tensor.matmul(s2_ps[:], w2_s, t1b[:], start=True, stop=True)
            nc.vector.tensor_copy(out=bias2[:], in_=s2_ps[:])
        else:
            h1t = hpool.tile([P, N], bf16, tag="h")
            nc.vector.tensor_scalar_add(out=h1t[:], in0=h1[:],
                                        scalar1=t1[:, 0:1])
            nc.tensor.matmul(z2_ps[:], w2_s, h1t[:], start=True, stop=True)
        # h0 block 0 (exact fp32; bias b0 rides in as the ones row) fills
        # the PE gap under relu2
        nc.tensor.matmul(h0_ps[:, 0, :], xt_s[:, 0:P], w0_s,
                         start=True, stop=True)
        h2 = hpool.tile([P, N], bf16, tag="h")
        t2 = stat.tile([P, 1], f32, tag="t")
        if bias_zero:
            nc.scalar.activation(h2[:], z2_ps[:], AF.Relu,
                                 bias=bias2[:, 0:1], accum_out=t2[:])
        else:
            nc.scalar.activation(h2[:], z2_ps[:], AF.Relu, bias=b1T_s,
                                 accum_out=t2[:])

        # ---- layer 3, node-major: z3[j] = (h2 + 1 t2^T)[:,j].T @ (c W3);
        # only this core's node half feeds it, the rest of h2 is consumed
        # through t2 alone
        h2t = hpool.tile([P, NH * H], bf16, tag="h2t")
        nc.vector.tensor_scalar_add(out=h2t[:, 0:H], in0=h2[:, 0:H],
                                    scalar1=t2[:, 0:1])
        nc.vector.tensor_scalar_add(out=h2t[:, H:2 * H], in0=h2[:, H:2 * H],
                                    scalar1=t2[:, 0:1])
        z3_ps = psumZ.tile([P, NH, H], f32, tag="z3")
        nc.tensor.matmul(z3_ps[:, 0, :], h2t[:, 0:P], w3_s,
                         start=True, stop=True)
        nc.tensor.matmul(z3_ps[:, 1, :], h2t[:, P:2 * P], w3_s,
                         start=True, stop=True)
        # h0 block 1 on the now-idle PE; its consumers (p1, feat copy) are
        # late in the tail
        nc.tensor.matmul(h0_ps[:, 1, :], xt_s[:, P:2 * P], w0_s,
                         start=True, stop=True)

        if bias_zero:
            ze = z3_ps  # exp straight off PSUM
        else:
            ze = work.tile([P, NH, H], f32, tag="z3b")
            nc.vector.tensor_add(out=ze[:, 0, :], in0=z3_ps[:, 0, :],
                                 in1=b2bc_s)
            nc.vector.tensor_add(out=ze[:, 1, :], in0=z3_ps[:, 1, :],
                                 in1=b2bc_s)

        # h0 -> SBUF copies (a tensor_tensor may read only ONE PSUM
        # operand, so the residual add needs h0 in SBUF): block 0 on the
        # scalar engine under the z3 matmuls, block 1 on the DVE right
        # before its use.
        h0b = work.tile([P, NH, H], f32, tag="h0b")
        nc.scalar.copy(out=h0b[:, 0, :], in_=h0_ps[:, 0, :])

        # log_softmax along the free axis + residual; values are O(+-10),
        # so exp() without max-subtraction is safe.  p = z3 + h0 overlaps
        # the exp; -lse = Ln(1/sum) so block 0 adds it as an activation
        # bias on scalar while block 1 adds it on DVE.
        e = work.tile([P, NH, H], bf16, tag="e")
        nc.scalar.activation(e[:], ze[:], AF.Exp)
        p = work.tile([P, NH, H], f32, tag="p")
        nc.vector.tensor_add(out=p[:, 0, :], in0=ze[:, 0, :],
                             in1=h0b[:, 0, :])
        ssum = stat.tile([P, NH], f32, tag="ssum")
        nc.vector.reduce_sum(ssum[:], e[:], axis=AX.X)
        rs = stat.tile([P, NH], f32, tag="rs")
        nc.vector.reciprocal(out=rs[:], in_=ssum[:])
        nc.vector.tensor_copy(out=h0b[:, 1, :], in_=h0_ps[:, 1, :])
        nc.vector.tensor_add(out=p[:, 1, :], in0=ze[:, 1, :],
                             in1=h0b[:, 1, :])
        nlse = stat.tile([P, NH], f32, tag="nlse")
        nc.scalar.activation(nlse[:], rs[:], AF.Ln)

        o = work.tile([P, NH, H], f32, tag="o")
        nc.scalar.activation(o[:, 0, :], p[:, 0, :], AF.Identity,
                             bias=nlse[:, 0:1])
        nc.scalar.dma_start(out=updh[0], in_=o[:, 0, :])
        nc.vector.tensor_scalar_add(out=o[:, 1, :], in0=p[:, 1, :],
                                    scalar1=nlse[:, 1:2])
        nc.sync.dma_start(out=updh[1], in_=o[:, 1, :])

        nc.gpsimd.dma_start(out=feath[:].rearrange("b p f -> p b f"),
                            in_=h0b[:])

        # consume the keep-alive PSUM so the filler matmuls survive DCE
        nc.vector.tensor_copy(out=warm[0:1, :], in_=ka_ps[0:1, 0:1])

    nc.compile()
    return nc


def _build_general_program():
    """Arbitrary edge_index: dense normalized adjacency as matmuls."""
    import concourse.mybir as mybir
    import concourse.tile as tile
    from concourse import bacc
    from contextlib import ExitStack

    f32 = mybir.dt.float32
    AF = mybir.ActivationFunctionType
    AX = mybir.AxisListType

    nc = bacc.Bacc("TRN2", target_bir_lowering=False, debug=False,
                   num_devices=NUM_CORES)

    xTp = nc.dram_tensor("xTp", [P, N], f32, kind="ExternalInput").ap()
    w0p = nc.dram_tensor("w0p", [P, H], f32, kind="ExternalInput").ap()
    b0T = nc.dram_tensor("b0T", [P, 1], f32, kind="ExternalInput").ap()
    b0bc = nc.dram_tensor("b0bc", [P, H], f32, kind="ExternalInput").ap()
    wsT = nc.dram_tensor("wsT", [P, L, H], f32, kind="ExternalInput").ap()
    bsT = nc.dram_tensor("bsT", [P, L], f32, kind="ExternalInput").ap()
    bs2bc = nc.dram_tensor("bs2bc", [P, H], f32, kind="ExternalInput").ap()
    at = nc.dram_tensor("at", [P, NB, N], f32, kind="ExternalInput").ap()

    upd = nc.dram_tensor("upd", [N, H], f32, kind="ExternalOutput").ap()
    feat = nc.dram_tensor("feat", [N, H], f32, kind="ExternalOutput").ap()

    with tile.TileContext(nc) as tc, ExitStack() as ctx:
        const = ctx.enter_context(tc.tile_pool(name="const", bufs=1))
        hpool = ctx.enter_context(tc.tile_pool(name="hpool", bufs=2))
        work = ctx.enter_context(tc.tile_pool(name="work", bufs=2))
        zpool = ctx.enter_context(tc.tile_pool(name="zpool", bufs=4))
        stat = ctx.enter_context(tc.tile_pool(name="stat", bufs=8))
        psum = ctx.enter_context(tc.tile_pool(name="psum", bufs=3, space="PSUM"))
        psumB = ctx.enter_context(tc.tile_pool(name="psumB", bufs=2, space="PSUM"))

        warm = stat.tile([P, 1], f32, tag="warm")
        nc.vector.memset(warm[:], 1.0)
        nc.scalar.activation(warm[:], warm[:], AF.Ln)

        xT_s = const.tile([P, N], f32)
        nc.sync.dma_start(out=xT_s[:], in_=xTp[:])
        w0_s = const.tile([P, H], f32)
        nc.sync.dma_start(out=w0_s[:], in_=w0p[:])
        ws_s = const.tile([P, L, H], f32)
        nc.sync.dma_start(out=ws_s[:], in_=wsT[:])
        b0T_s = const.tile([P, 1], f32)
        nc.sync.dma_start(out=b0T_s[:], in_=b0T[:])
        bsT_s = const.tile([P, L], f32)
        nc.sync.dma_start(out=bsT_s[:], in_=bsT[:])
        b0bc_s = const.tile([P, H], f32)
        nc.sync.dma_start(out=b0bc_s[:], in_=b0bc[:])
        bs2bc_s = const.tile([P, H], f32)
        nc.sync.dma_start(out=bs2bc_s[:], in_=bs2bc[:])
        at_s = const.tile([P, NB, N], f32)
        nc.sync.dma_start(out=at_s[:], in_=at[:])

        h0T_ps = psumB.tile([P, N], f32, tag="big")
        nc.tensor.matmul(h0T_ps[:], w0_s[:], xT_s[:], start=True, stop=True)
        hT = hpool.tile([P, N], f32, tag="hT")
        nc.vector.tensor_scalar_add(out=hT[:], in0=h0T_ps[:],
                                    scalar1=b0T_s[:, 0:1])

        h0_s = const.tile([P, NB, H], f32)
        for b in range(NB):
            ps = psum.tile([P, H], f32, tag="mm")
            nc.tensor.matmul(ps[:], xT_s[:, b * P:(b + 1) * P], w0_s[:],
                             start=True, stop=True)
            nc.vector.tensor_add(out=h0_s[:, b, :], in0=ps[:], in1=b0bc_s[:])
            nc.sync.dma_start(out=feat[b * P:(b + 1) * P, :], in_=h0_s[:, b, :])

        for l in range(L):
            hw_s = work.tile([P, NB, H], f32, tag="hw")
            for b in range(NB):
                ps = psum.tile([P, H], f32, tag="mm")
                nc.tensor.matmul(ps[:], hT[:, b * P:(b + 1) * P],
                                 ws_s[:, l, :], start=True, stop=True)
                nc.vector.tensor_copy(out=hw_s[:, b, :], in_=ps[:])

            if l < L - 1:
                aggT_ps = psumB.tile([P, N], f32, tag="big")
                for cc in range(NB):
                    nc.tensor.matmul(aggT_ps[:], hw_s[:, cc, :], at_s[:, cc, :],
                                     start=(cc == 0), stop=(cc == NB - 1))
                hT_new = hpool.tile([P, N], f32, tag="hT")
                nc.scalar.activation(hT_new[:], aggT_ps[:], AF.Relu,
                                     bias=bsT_s[:, l:l + 1])
                hT = hT_new
            else:
                z_s = []
                negm_s = []
                s_sum = stat.tile([P, NB], f32, tag="ssum")
                for b in range(NB):
                    agg_ps = psum.tile([P, H], f32, tag="mm")
                    for cc in range(NB):
                        nc.tensor.matmul(agg_ps[:],
                                         at_s[:, cc, b * P:(b + 1) * P],
                                         hw_s[:, cc, :],
                                         start=(cc == 0), stop=(cc == NB - 1))
                    z = zpool.tile([P, H], f32, tag="z")
                    nc.vector.tensor_add(out=z[:], in0=agg_ps[:], in1=bs2bc_s[:])
                    negm = stat.tile([P, 1], f32, tag="negm")
                    nc.vector.reduce_max(negm[:], z[:], axis=AX.X, negate=True)
                    z_s.append(z)
                    negm_s.append(negm)
                for b in range(NB):
                    e = zpool.tile([P, H], f32, tag="e")
                    nc.scalar.activation(e[:], z_s[b][:],
                                         mybir.ActivationFunctionType.Exp,
                                         bias=negm_s[b][:, 0:1],
                                         accum_out=s_sum[:, b:b + 1])
                lse = stat.tile([P, NB], f32, tag="lse")
                nc.scalar.activation(lse[:], s_sum[:],
                                     mybir.ActivationFunctionType.Ln)
                for b in range(NB):
                    tot = stat.tile([P, 1], f32, tag="tot")
                    nc.vector.tensor_sub(out=tot[:], in0=lse[:, b:b + 1],
                                         in1=negm_s[b][:])
                    o = zpool.tile([P, H], f32, tag="o")
                    nc.vector.scalar_tensor_tensor(
                        out=o[:], in0=z_s[b][:], scalar=tot[:, 0:1],
                        in1=h0_s[:, b, :],
                        op0=mybir.AluOpType.subtract, op1=mybir.AluOpType.add)
                    nc.sync.dma_start(out=upd[b * P:(b + 1) * P, :], in_=o[:])

    nc.compile()
    return nc


def _edge_structure(edge_index: np.ndarray):
    """Return True iff edge_index is exactly all-pairs + one self loop per
    node (uniform deg = N+1)."""
    src = edge_index[0].astype(np.int64)
    dst = edge_index[1].astype(np.int64)
    if src.shape[0] != N * N + N:
        return False
    if src.min() < 0 or src.max() >= N or dst.min() < 0 or dst.max() >= N:
        return False
    counts = np.bincount(src * N + dst, minlength=N * N).reshape(N, N)
    expect = np.ones((N, N), dtype=counts.dtype)
    np.fill_diagonal(expect, 2)
    return np.array_equal(counts, expect)


def _build_adjacency(edge_index: np.ndarray) -> np.ndarray:
    """Dense normalized adjacency, transposed: AT[src, dst] (= A.T)."""
    src = edge_index[0].astype(np.int64)
    dst = edge_index[1].astype(np.int64)
    deg = np.bincount(dst, minlength=N).astype(np.float32)
    dinv = np.where(deg > 0, 1.0 / np.sqrt(deg), 0.0).astype(np.float32)
    norm = (dinv[src] * dinv[dst]).astype(np.float32)
    at = np.bincount(src * N + dst, weights=norm.astype(np.float64),
                     minlength=N * N).reshape(N, N)
    return at.astype(np.float32)


def _pad_rows(a: np.ndarray, rows: int) -> np.ndarray:
    out = np.zeros((rows,) + a.shape[1:], dtype=a.dtype)
    out[:a.shape[0]] = a
    return out


def _structured_packs(x, W0, b0, Ws, bs, bias_zero=False):
    """Host-side constant folding (weights in fp64, rounded once)."""
    import ml_dtypes
    bf = ml_dtypes.bfloat16
    c = 1.0 / (N + 1)
    W0_64 = W0.astype(np.float64)
    Ws_64 = Ws.astype(np.float64)
    G = c * (W0_64 @ Ws_64[0])                       # [2, H]
    row1 = Ws_64[0].T @ b0.astype(np.float64) + bs[0]  # [H]

    bpk = np.zeros((P, 1 + H), dtype=bf)
    bpk[:, 0] = bs[1].astype(bf)
    bpk[:, 1:] = np.broadcast_to(bs[2], (P, H)).astype(bf)

    wpk = np.zeros((P, 2 * H), dtype=bf)
    wpk[:, :H] = (c * Ws_64[1]).astype(bf)
    wpk[:, H:] = (c * Ws_64[2]).astype(bf)

    shared = {"wpk": wpk} if bias_zero else {"bpk": bpk, "wpk": wpk}

    per_core = []
    for core in range(NUM_CORES):
        g = core % B
        half = core // B
        hn = N // 2
        mine = np.arange(half * hn, (half + 1) * hn)
        other = np.arange(0, half * hn)
        rest = np.arange((half + 1) * hn, N)
        perm = np.concatenate([mine, other, rest])  # this core's half first
        x64 = x[g].astype(np.float64)
        xs2 = (x64 + x64.sum(0))[perm]               # [N, 2]

        bfin = np.zeros((32, _BF_COLS), dtype=bf)
        bfin[:D, _BF_G:_BF_G + H] = G.astype(bf)
        bfin[D, _BF_G:_BF_G + H] = row1.astype(bf)
        bfin[:D, _BF_X:_BF_X + N] = xs2.T.astype(bf)
        bfin[D, _BF_X:_BF_X + N] = 1.0

        xpk = np.zeros((32, _XP_COLS), dtype=np.float32)
        xpk[:D, _XP_XT:_XP_XT + N] = x[g][perm].T
        xpk[D, _XP_XT:_XP_XT + N] = 1.0
        xpk[:D, _XP_W0:_XP_W0 + H] = W0
        xpk[D, _XP_W0:_XP_W0 + H] = b0
        m = dict(shared)
        m["bfin"] = bfin
        m["xpk"] = xpk
        per_core.append(m)
    return per_core


def kernel(x, W0, b0, Ws, bs, edge_index):
    from concourse.bass_utils import run_bass_kernel_spmd

    _patch_act_tables()

    x = np.ascontiguousarray(np.asarray(x, dtype=np.float32))
    W0 = np.ascontiguousarray(np.asarray(W0, dtype=np.float32))
    b0 = np.ascontiguousarray(np.asarray(b0, dtype=np.float32))
    Ws = np.ascontiguousarray(np.asarray(Ws, dtype=np.float32))
    bs = np.ascontiguousarray(np.asarray(bs, dtype=np.float32))
    edge_index = np.asarray(edge_index, dtype=np.int32)

    structured = _edge_structure(edge_index)
    if structured:
        bias_zero = not (b0.any() or bs.any())
        in_maps = _structured_packs(x, W0, b0, Ws, bs, bias_zero)
        key = ("structured", bias_zero)
        if key not in _PROGRAM_CACHE:
            _PROGRAM_CACHE[key] = _build_structured_program(bias_zero)
        nc = _PROGRAM_CACHE[key]
    else:
        shared = {
            "w0p": _pad_rows(W0, P),
            "b0T": np.ascontiguousarray(b0.reshape(P, 1)),
            "b0bc": np.ascontiguousarray(np.broadcast_to(b0, (P, H))),
            "wsT": np.ascontiguousarray(Ws.transpose(1, 0, 2)),
            "bsT": np.ascontiguousarray(bs.T),
        }
        key = "general"
        if key not in _PROGRAM_CACHE:
            _PROGRAM_CACHE[key] = _build_general_program()
        nc = _PROGRAM_CACHE[key]
        at = _build_adjacency(edge_index)
        shared["at"] = np.ascontiguousarray(
            at.reshape(NB, P, N).transpose(1, 0, 2))
        shared["bs2bc"] = np.ascontiguousarray(
            np.broadcast_to(bs[L - 1], (P, H)))
        in_maps = []
        for core in range(NUM_CORES):
            g = core % B
            m = dict(shared)
            m["xTp"] = _pad_rows(np.ascontiguousarray(x[g].T), P)
            in_maps.append(m)

    res = run_bass_kernel_spmd(nc, in_maps, list(range(NUM_CORES)))
    _PROGRAM_CACHE["last_results"] = res

    if structured:
        upd = np.empty((B, N, H), dtype=np.float32)
        feat = np.empty((B, N, H), dtype=np.float32)
        for core in range(NUM_CORES):
            g = core % B
            half = core // B
            sl = slice(half * (N // 2), (half + 1) * (N // 2))
            upd[g, sl] = res.results[core]["updh"].reshape(N // 2, H)
            feat[g, sl] = res.results[core]["feath"].reshape(N // 2, H)
    else:
        upd = np.stack([res.results[g]["upd"] for g in range(B)])
        feat = np.stack([res.results[g]["feat"] for g in range(B)])
    return upd, feat
